# revision 1
# baseline (speedup 1.0000x reference)
"""CBAM kernel for Trainium2, data-parallel over batch across 8 NeuronCores.

Per core: x_loc [4, 256, 64, 64] f32 viewed as 8 tiles [128, 4096]
(tile t: batch b = t//2, channel half = t%2; free dim = h*64+w).

Phases (per core):
  A: per-(b,c) sum (ACT activation-accumulate) and max (DVE reduce) over HW.
  MLP: channel attention on PE (w1/w2 pre-transposed on host; the /HW mean
      normalization is folded into w1_avg; relu is positively homogeneous).
  C: per b: m = max(x_t0*s0, x_t1*s1) (DVE tensor_scalar + scalar_tensor_tensor),
     cross-channel max via PE 128x128 transposes + DVE 3D-AP reduce,
     cross-channel weighted sum via PE matmul with lhsT = ch_att column,
     both folded into a [32 k, 70 wpad] per-(b, map, delta) layout (h = 2k+delta).
  conv: 7x7/2ch SAME conv as 28 accumulated PE matmuls with host-precomputed
     banded Toeplitz lhsT (column decomposition; h-conv in the contraction,
     w-shift via rhs AP offset into the zero-padded strips).
  D: sigmoid (ACT), fold rows to flat [1, 4096] (SBUF DMA), broadcast to all
     128 partitions via PE rank-1 matmul, final out = (x*s)*g fused on DVE
     scalar_tensor_tensor, in-place over x, DMA out.
"""

import numpy as np

C = 256
R = 16
K7 = 7
B_GLOBAL = 32
N_CORES = 8
B_LOC = B_GLOBAL // N_CORES  # 4
H = W = 64
HW = H * W  # 4096
NT = B_LOC * C // 128  # 8 tiles per core
CH = C // R  # 16 hidden


def _apply_tile_drain_patch():
    """This container's walrus rejects instructions carrying >1 semaphore wait
    (CoreV3 setupSyncWait "Too many sync wait commands"). TileContext's exit
    drain attaches one wait per live semaphore to a single Drain instruction;
    split the extras onto standalone sync-engine NOPs (one wait each)."""
    import bass_rust
    import concourse.tile as tile
    from concourse.vector_clock import ScopedClock

    if getattr(tile.TileContext, "_drain_split_patch", False):
        return

    def _drain_and_barrier_split(self, tick_clock, wait_clock):
        nc = self.nc
        drain_inst = nc.sync.drain()
        wait_clock.add_sem_waits(
            drain_inst.ins, ScopedClock({None: tick_clock.global_clock})
        )
        si = drain_inst.ins.sync_info
        if si is not None and si.on_wait is not None and len(si.on_wait) > 1:
            extras = list(si.on_wait[1:])
            del si.on_wait[1:]
            for w in extras:
                nop = nc.sync.nop(nofuse=True)
                if nop.ins.sync_info is None:
                    nop.ins.sync_info = bass_rust.SyncInfo(on_wait=[w], on_update=[])
                else:
                    nop.ins.sync_info.on_wait.append(w)

        nc.all_engine_barrier()
        assert self.sems is not None
        popped = nc._tile_sem_poison_stack.pop()
        assert popped is self._sem_poison
        nc.clear_and_free_semaphores(list(self.sems.allocated().values()))
        nc.all_engine_barrier()

    tile.TileContext._drain_and_barrier = _drain_and_barrier_split
    tile.TileContext._drain_split_patch = True


def _split_multiwait(nc):
    """Walrus here encodes at most ONE semaphore wait per instruction
    (CoreV2/V3 setupSyncWait: "Too many sync wait commands"); Tile's wait
    assignment can attach several. Keep one wait on the instruction and hoist
    the rest onto same-engine NOPs inserted right before it."""
    from concourse import mybir

    for bb in nc.m.functions[0].blocks:
        out = []
        for ins in bb.instructions:
            si = ins.sync_info
            if si is not None and si.on_wait is not None and len(si.on_wait) > 1:
                extras = list(si.on_wait[:-1])
                del si.on_wait[:-1]
                for k, w in enumerate(extras):
                    out.append(mybir.InstNoOp(
                        name=f"{ins.name}_sw{k}",
                        engine=ins.engine,
                        sync_info=mybir.SyncInfo(on_wait=[w], on_update=[]),
                        bass_nofuse=True,
                    ))
            out.append(ins)
        bb.instructions[:] = out


def build_nc():
    _apply_tile_drain_patch()
    import concourse.bass as bass
    import concourse.tile as tile
    from concourse import mybir
    from concourse.masks import make_identity

    f32 = mybir.dt.float32
    f32r = mybir.dt.float32  # f32r reverted: ISA checks reject M=1/K=1 fp32r matmuls here
    AF = mybir.ActivationFunctionType
    OP = mybir.AluOpType
    AX = mybir.AxisListType

    nc = bass.Bass()
    x_d = nc.declare_dram_parameter("x", [B_LOC * C, HW], f32, isOutput=False)
    w1a_d = nc.declare_dram_parameter("w1t_avg", [C, CH], f32, isOutput=False)
    w1m_d = nc.declare_dram_parameter("w1t_max", [C, CH], f32, isOutput=False)
    w2t_d = nc.declare_dram_parameter("w2t", [CH, C], f32, isOutput=False)
    bc_d = nc.declare_dram_parameter("bconv", [32, 28 * 64], f32, isOutput=False)
    out_d = nc.declare_dram_parameter("out", [B_LOC * C, HW], f32, isOutput=True)

    x_t = x_d[:].rearrange("(t p) f -> t p f", p=128)
    out_t = out_d[:].rearrange("(t p) f -> t p f", p=128)
    w1a_t = w1a_d[:].rearrange("(u p) r -> u p r", p=128)
    w1m_t = w1m_d[:].rearrange("(u p) r -> u p r", p=128)

    def r(ap):
        # fp32 -> fp32r reinterpret: PE runs fp32r matmuls at full rate for
        # moving dims >= 256 (fp32 pays 4 cycles/row)
        return ap.bitcast(f32r)

    with tile.TileContext(nc) as tc:
        with (
            tc.tile_pool(name="xp", bufs=1) as xp,
            tc.tile_pool(name="mp", bufs=2) as mp,
            tc.tile_pool(name="mfp", bufs=2) as mfp,
            tc.tile_pool(name="sm", bufs=1) as sm,
            tc.tile_pool(name="ptrans", bufs=2, space="PSUM") as ptrans,
            tc.tile_pool(name="pbcast", bufs=2, space="PSUM") as pbcast,
            tc.tile_pool(name="psmall", bufs=2, space="PSUM") as psmall,
        ):
            # ---------- constants / small tiles ----------
            # weight DMAs FIRST on HWDGE so they don't queue behind the 2 MiB
            # x-streams (the MLP needs them early)
            w1a = [sm.tile([128, CH], f32, tag=f"w1a{u}", name=f"w1a{u}") for u in range(2)]
            w1m = [sm.tile([128, CH], f32, tag=f"w1m{u}", name=f"w1m{u}") for u in range(2)]
            w2t = sm.tile([CH, C], f32, tag="w2t")
            bconv = sm.tile([32, 28 * 64], f32r, tag="bconv")
            for u in range(2):
                nc.sync.dma_start(out=w1a[u][:], in_=w1a_t[u])
                nc.sync.dma_start(out=w1m[u][:], in_=w1m_t[u])
            nc.sync.dma_start(out=w2t[:], in_=w2t_d[:])
            nc.sync.dma_start(out=bconv[:], in_=bc_d[:].bitcast(f32r))

            ident = sm.tile([128, 128], f32, tag="ident")
            make_identity(nc, ident[:])
            ident_r = sm.tile([128, 128], f32r, tag="ident_r")
            nc.vector.tensor_copy(out=ident_r[:], in_=ident[:])
            ones4 = sm.tile([1, 128], f32r, tag="ones4")
            nc.vector.memset(ones4[:].bitcast(f32), 1.0)
            a1sink = sm.tile([128, 1], f32, tag="a1sink")

            sum_stat = sm.tile([128, NT], f32, tag="sum_stat")
            max_stat = sm.tile([128, NT], f32, tag="max_stat")
            chatt = sm.tile([128, NT], f32r, tag="chatt")
            hrelu_a = sm.tile([CH, 2 * B_LOC], f32, tag="hrelu_a")
            # [k, b, strip(=ch*2+delta), wpad]
            stile = sm.tile([32, B_LOC, 4, 70], f32r, tag="stile")
            zsrc = sm.tile([32, B_LOC, 4, 70], f32, tag="zsrc")
            nc.vector.memset(zsrc[:], 0.0)
            nc.vector.tensor_copy(out=stile[:], in_=zsrc[:])
            attsig = sm.tile([64, B_LOC, 64], f32r, tag="attsig")

            xs = []
            for t in range(NT):
                xt = xp.tile([128, HW], f32r, tag=f"x{t}", name=f"x{t}")
                xs.append(xt)

            # ---------- fully per-b pipeline (no global barriers) ----------
            for b in range(B_LOC):
                t0, t1 = 2 * b, 2 * b + 1
                # -- load + per-tile stats (sum via in-place ACT Copy accum,
                #    max via DVE reduce) --
                for t in (t0, t1):
                    xt = xs[t]
                    nc.sync.dma_start(out=xt[:], in_=x_t[t].bitcast(f32r))
                    nc.scalar.activation(
                        out=a1sink.to_broadcast([128, HW]),
                        in_=xt[:].bitcast(f32),
                        func=AF.Copy, accum_out=sum_stat[:, t : t + 1],
                    )
                    nc.vector.reduce_max(
                        out=max_stat[:, t : t + 1], in_=xt[:].bitcast(f32),
                        axis=AX.X
                    )

                # -- per-b MLP (channel attention); h_a/h_m share one psum
                #    tile so only one "ps" slot is held --
                h = psmall.tile([CH, 2], f32, tag="ps", name="h")
                nc.tensor.matmul(h[:, 0:1], lhsT=w1a[0][:],
                                 rhs=sum_stat[:, t0 : t0 + 1],
                                 start=True, stop=False)
                nc.tensor.matmul(h[:, 0:1], lhsT=w1a[1][:],
                                 rhs=sum_stat[:, t1 : t1 + 1],
                                 start=False, stop=True)
                nc.tensor.matmul(h[:, 1:2], lhsT=w1m[0][:],
                                 rhs=max_stat[:, t0 : t0 + 1],
                                 start=True, stop=False)
                nc.tensor.matmul(h[:, 1:2], lhsT=w1m[1][:],
                                 rhs=max_stat[:, t1 : t1 + 1],
                                 start=False, stop=True)
                hrelu = hrelu_a  # [CH, B_LOC*2]: cols (b, branch)
                nc.scalar.activation(out=hrelu[:, 2 * b : 2 * b + 2], in_=h[:],
                                     func=AF.Relu)
                for half in range(2):
                    o = psmall.tile([128, 1], f32, tag="ps", name="mlp_o")
                    w2s = w2t[:, 128 * half : 128 * (half + 1)]
                    nc.tensor.matmul(o[:], lhsT=w2s,
                                     rhs=hrelu[:, 2 * b : 2 * b + 1],
                                     start=True, stop=False)
                    nc.tensor.matmul(o[:], lhsT=w2s,
                                     rhs=hrelu[:, 2 * b + 1 : 2 * b + 2],
                                     start=False, stop=True)
                    tt = 2 * b + half
                    nc.scalar.activation(out=chatt[:, tt : tt + 1],
                                         in_=o[:], func=AF.Sigmoid)

                s0 = chatt[:, t0 : t0 + 1]          # f32r (matmul lhsT)
                s1 = chatt[:, t1 : t1 + 1]
                s0f = s0.bitcast(f32)               # fp32 view (DVE/ACT)
                s1f = s1.bitcast(f32)

                # -- spatial maps: m = max(x0*s0, x1*s1) (scale on ACT, max
                #    on DVE STT) --
                m = mp.tile([128, HW], f32r, tag="m", name="m")
                nc.scalar.activation(out=m[:], in_=xs[t0][:].bitcast(f32),
                                     func=AF.Copy, scale=s0f)
                nc.vector.scalar_tensor_tensor(
                    out=m[:], in0=xs[t1][:].bitcast(f32), scalar=s1f,
                    in1=m[:].bitcast(f32), op0=OP.mult, op1=OP.max,
                )
                mfold = mfp.tile([128, 32], f32r, tag="mf", name="mfold")
                for f in range(8):  # 8 fills of 4 transposed 128x128 blocks
                    ptr = ptrans.tile([128, 512], f32, tag="ptr", name="ptr")
                    for j in range(4):
                        blk = 4 * f + j
                        nc.tensor.transpose(
                            ptr[:, 128 * j : 128 * (j + 1)].bitcast(f32r),
                            m[:, 128 * blk : 128 * (blk + 1)],
                            ident_r[:],
                        )
                    nc.vector.reduce_max(
                        out=mfold[:, 4 * f : 4 * f + 4],
                        in_=ptr[:].rearrange("p (k c) -> p k c", c=128),
                        axis=AX.X,
                    )
                # -- weighted channel-sum (mean*C): 8 hw-chunks, grouped by
                #    lhsT to avoid weight reloads; psum rows only at
                #    {0,32,64} -> 3 tiles --
                meanrow = mfp.tile([128, 1024], f32, tag="meanrow",
                                   name="meanrow")
                groups = [(0, 1, 2), (3, 4, 5), (6, 7)]
                for grp in groups:
                    pm = psmall.tile([128, 512], f32, tag="ps", name="pm")
                    for s_col, xt, st in ((s0, xs[t0], True), (s1, xs[t1], False)):
                        for j, f in enumerate(grp):
                            sl = slice(512 * f, 512 * (f + 1))
                            nc.tensor.matmul(
                                pm[32 * j : 32 * j + 1, :], lhsT=s_col,
                                rhs=xt[:, sl], start=st, stop=not st)
                    for j, f in enumerate(grp):
                        cg, jj = f // 4, f % 4
                        nc.scalar.activation(
                            out=meanrow[32 * jj : 32 * jj + 1,
                                        512 * cg : 512 * (cg + 1)],
                            in_=pm[32 * j : 32 * j + 1, :], func=AF.Copy,
                        )
                for cg in range(2):
                    src = meanrow[0:128:32, 512 * cg : 512 * (cg + 1)]
                    nc.gpsimd.dma_start(
                        out=stile[16 * cg : 16 * (cg + 1), b, 0:2, 3:67],
                        in_=src.rearrange("p (k d w) -> p k d w",
                                          d=2, w=64).bitcast(f32r),
                    )
                # -- fold max map: M[64d + w, k] -> stile[k, b, 2+d, 3+w] --
                for d in range(2):
                    pt = psmall.tile([32, 64], f32, tag="ps", name="pt")
                    nc.tensor.transpose(
                        pt[:].bitcast(f32r),
                        mfold[64 * d : 64 * (d + 1), :],
                        ident_r[64 * d : 64 * (d + 1), 64 * d : 64 * (d + 1)],
                    )
                    nc.scalar.activation(
                        out=stile[:, b, 2 + d, 3:67], in_=pt[:], func=AF.Copy
                    )

                # -- per-b conv (28 accumulated matmuls) + sigmoid --
                pconv = psmall.tile([64, 64], f32, tag="ps", name="pconv")
                first = True
                for ch in range(2):
                    for kw in range(K7):
                        for d in range(2):
                            combo = (ch * K7 + kw) * 2 + d
                            strip = ch * 2 + d
                            nc.tensor.matmul(
                                pconv[:],
                                lhsT=bconv[:, 64 * combo : 64 * (combo + 1)],
                                rhs=stile[:, b, strip, kw : kw + 64],
                                start=first,
                                stop=(combo == 27),
                            )
                            first = False
                nc.scalar.activation(out=attsig[:, b, :], in_=pconv[:],
                                     func=AF.Sigmoid)

                # -- broadcast + final fused multiply + store --
                attflat = mp.tile([1, HW], f32r, tag="m", name="attflat")
                nc.gpsimd.dma_start(out=attflat[:], in_=attsig[:, b, :])
                for cchunk in range(4):
                    pb = pbcast.tile([128, 1024], f32, tag="pb", name="pb")
                    for s in range(2):
                        lo = 1024 * cchunk + 512 * s
                        nc.tensor.matmul(
                            pb[:, 512 * s : 512 * (s + 1)],
                            lhsT=ones4[0:1, :],
                            rhs=attflat[0:1, lo : lo + 512],
                            start=True, stop=True,
                        )
                    sl = slice(1024 * cchunk, 1024 * (cchunk + 1))
                    nc.vector.scalar_tensor_tensor(
                        out=xs[t0][:, sl],
                        in0=xs[t0][:, sl].bitcast(f32), scalar=s0f,
                        in1=pb[:], op0=OP.mult, op1=OP.mult,
                    )
                    nc.sync.dma_start(out=out_t[t0][:, sl],
                                      in_=xs[t0][:, sl].bitcast(f32))
                    nc.vector.scalar_tensor_tensor(
                        out=xs[t1][:, sl],
                        in0=xs[t1][:, sl].bitcast(f32), scalar=s1f,
                        in1=pb[:], op0=OP.mult, op1=OP.mult,
                    )
                    nc.sync.dma_start(out=out_t[t1][:, sl],
                                      in_=xs[t1][:, sl].bitcast(f32))

    return nc


def make_bconv(w_sp):
    """bconv[k, combo*64 + h_out] with combo = (ch*7 + kw)*2 + delta,
    B[k, h] = w_sp[0, ch, 2k + delta - h + 3, kw] (0 outside), /C for ch=0."""
    w_sp = np.asarray(w_sp, np.float32)
    out = np.zeros((32, 28 * 64), np.float32)
    k = np.arange(32)[:, None]
    h = np.arange(64)[None, :]
    for ch in range(2):
        for kw in range(K7):
            for d in range(2):
                combo = (ch * K7 + kw) * 2 + d
                a = 2 * k + d - h + 3
                valid = (a >= 0) & (a < K7)
                vals = np.where(valid, w_sp[0, ch, np.clip(a, 0, K7 - 1), kw], 0.0)
                if ch == 0:
                    vals = vals / C
                out[:, 64 * combo : 64 * (combo + 1)] = vals
    return out


_CACHED = {}


def _get_nc():
    if "nc" not in _CACHED:
        nc = build_nc()
        _split_multiwait(nc)
        _CACHED["nc"] = nc
    return _CACHED["nc"]


def make_in_maps(x, w1, w2, w_sp):
    x = np.ascontiguousarray(np.asarray(x, np.float32))
    w1 = np.asarray(w1, np.float32)
    w2 = np.asarray(w2, np.float32)
    w1t_avg = np.ascontiguousarray((w1 / HW).T)
    w1t_max = np.ascontiguousarray(w1.T)
    w2t = np.ascontiguousarray(w2.T)
    bconv = make_bconv(w_sp)
    in_maps = []
    for c in range(N_CORES):
        xl = x[B_LOC * c : B_LOC * (c + 1)].reshape(B_LOC * C, HW)
        in_maps.append({
            "x": np.ascontiguousarray(xl),
            "w1t_avg": w1t_avg,
            "w1t_max": w1t_max,
            "w2t": w2t,
            "bconv": bconv,
        })
    return in_maps


def kernel(x, w1, w2, w_sp):
    from concourse.bass_utils import run_bass_kernel_spmd

    nc = _get_nc()
    in_maps = make_in_maps(x, w1, w2, w_sp)
    res = run_bass_kernel_spmd(nc, in_maps, list(range(N_CORES)))
    outs = [
        res.results[c]["out"].reshape(B_LOC, C, H, W) for c in range(N_CORES)
    ]
    return np.concatenate(outs, 0)

